# revision 1
# baseline (speedup 1.0000x reference)
"""Euler integrator (low-rank quadratic Christoffel term) on 8 trn2 NeuronCores.

Math: per step   h = v @ U; gamma = (h*h) @ W; v' = v + dt*(force - gamma);
                 x' = wrap(x + dt*v)
Key reduction: the dynamics close in the rank-64 space:
    h_{t+1} = h_t + dt*(force@U) - (h_t^2) @ (dt*W@U)
    v_T = v_0 + T*dt*force - dt * A @ W,          A = sum_t h_t^2
    x_T = wrap(x_0 + T*dt*v_0 + 28*dt^2*force - dt^2 * G @ W),
                                                  G = sum_t (T-1-t) h_t^2
with T=8.  One wrap at the end (mod-2pi commutes with addition), so HBM
traffic is the 5-tensor roofline.  All PE operands are bf16 (fp32 matmuls
double-pump via LOW_HIGH); exact fp32 +x0/+v0 adds happen on DVE.  Running h
lives in PSUM, updated purely by accumulating matmuls; transposes for the
rank-64 projection go through the DMA xbar (bf16).
"""

import sys

sys.path.insert(0, "/opt/trn_rl_repo")

import numpy as np
import ml_dtypes

import concourse.bacc as bacc
import concourse.mybir as mybir
import concourse.tile as tile
from concourse.tile_rust import add_dep_helper
from concourse.bass_utils import run_bass_kernel_spmd

F32 = mybir.dt.float32
BF16 = mybir.dt.bfloat16

DT = 0.01
PI = float(np.pi)
TWO_PI = 2.0 * PI
B, D, R = 262144, 256, 64
NCORES = 8
BL = B // NCORES          # rows per core
STEPS = 8
PACK = 1024               # batch rows per pack
NBLK = PACK // 128        # natural 128-row blocks per pack (8)
HN = 512                  # free size of h-space tiles (PACK/2)
MAGIC = 12582912.0        # 1.5 * 2**23 (fp32 RNE rounding trick)


def _chain(*insts):
    for a, b in zip(insts[1:], insts[:-1]):
        add_dep_helper(a.ins, b.ins, sync=True, reason="psum group order")


def _build(bl: int):
    npack = bl // PACK
    nc = bacc.Bacc("TRN2", target_bir_lowering=False, debug=False)

    xg = nc.declare_dram_parameter("xg", [bl, D], F32, isOutput=False)
    vg = nc.declare_dram_parameter("vg", [bl, D], F32, isOutput=False)
    fg = nc.declare_dram_parameter("fg", [bl, D], F32, isOutput=False)
    # constants (host-prepared, tiny; all bf16 for single-pass matmuls)
    cdefs = {
        "u0z": 128, "u1z": 128,     # [U0|0], [U1|0]
        "u0": R, "u1": R,           # U halves
        "mdn": R,                   # -dt*(W@U), dup'd on both partition halves
        "i64": R,                   # I_64 dup'd
        "wn": D, "wnn": D,          # -dt*W, -dt^2*W, dup'd
        "i128": 128,                # I_128 (A accumulation)
        "if8": 128, "i28": 128,     # 8dt*I, 28dt^2*I
    }
    cdram = {
        nm: nc.declare_dram_parameter(nm, [128, w], BF16, isOutput=False)
        for nm, w in cdefs.items()
    }
    xo = nc.declare_dram_parameter("xo", [bl, D], F32, isOutput=True)
    vo = nc.declare_dram_parameter("vo", [bl, D], F32, isOutput=True)

    A = mybir.AluOpType

    with tile.TileContext(nc) as tc:
        with (
            tc.tile_pool(name="consts", bufs=1) as cpool,
            tc.tile_pool(name="nat", bufs=2) as nat,
            tc.tile_pool(name="natx", bufs=2) as natx,
            tc.tile_pool(name="natb", bufs=2) as natb,
            tc.tile_pool(name="trans", bufs=2) as trans,
            tc.tile_pool(name="hsp", bufs=3) as hsp,
            tc.tile_pool(name="acc", bufs=2) as accp,
            tc.tile_pool(name="atp", bufs=2) as atp,
            tc.tile_pool(name="outp", bufs=2) as outp,
            tc.tile_pool(name="wrapp", bufs=2) as wrapp,
            tc.tile_pool(name="ph", bufs=2, space="PSUM") as php,
            tc.tile_pool(name="pf", bufs=1, space="PSUM") as pfp,
            tc.tile_pool(name="pA", bufs=2, space="PSUM") as pAp,
            tc.tile_pool(name="pe", bufs=3, space="PSUM") as pep,
        ):
            cs = {}
            for nm, w in cdefs.items():
                t_ = cpool.tile([128, w], BF16, tag=nm)
                nc.sync.dma_start(out=t_[:], in_=cdram[nm][:])
                cs[nm] = t_
            magic_s = cpool.tile([128, 1], F32, tag="magic")
            nc.vector.memset(magic_s[:], MAGIC)

            for p in range(npack):
                rows = slice(p * PACK, (p + 1) * PACK)

                # ---- load v, force natural fp32; cast to bf16
                vt = nat.tile([128, NBLK, D], F32, tag="vt")
                ft = nat.tile([128, NBLK, D], F32, tag="ft")
                nc.sync.dma_start(
                    out=vt[:], in_=vg[rows, :].rearrange("(n p) d -> p n d", p=128)
                )
                nc.sync.dma_start(
                    out=ft[:], in_=fg[rows, :].rearrange("(n p) d -> p n d", p=128)
                )
                vb = natb.tile([128, NBLK, D], BF16, tag="vb")
                fb = natb.tile([128, NBLK, D], BF16, tag="fb")
                nc.scalar.copy(vb[:], vt[:])
                nc.scalar.copy(fb[:], ft[:])

                # ---- transpose vb, fb -> [128(d-chunk), 1024(b)] via DMA xbar
                vT0 = trans.tile([128, PACK], BF16, tag="vT0")
                vT1 = trans.tile([128, PACK], BF16, tag="vT1")
                fT0 = trans.tile([128, PACK], BF16, tag="fT0")
                fT1 = trans.tile([128, PACK], BF16, tag="fT1")
                for src, dsts in ((vb, (vT0, vT1)), (fb, (fT0, fT1))):
                    for dch in range(2):
                        for blk in range(NBLK):
                            nc.sync.dma_start(
                                out=dsts[dch][:, blk * 128:(blk + 1) * 128],
                                in_=src[:, blk, dch * 128:(dch + 1) * 128],
                                transpose=True,
                            )

                # ---- h0 into persistent psum bank; fU -> fUdt (bf16)
                ph = php.tile([128, HN], F32, tag="ph")
                pf = pfp.tile([128, HN], F32, tag="pf")
                for bank, t0, t1 in ((ph, vT0, vT1), (pf, fT0, fT1)):
                    _chain(
                        nc.tensor.matmul(
                            bank[:, :], cs["u0z"][:], t0[:, 0:HN],
                            start=True, stop=False,
                        ),
                        nc.tensor.matmul(
                            bank[64:128, :], cs["u0"][:], t0[:, HN:PACK],
                            start=False, stop=False, skip_group_check=True,
                        ),
                        nc.tensor.matmul(
                            bank[64:128, :], cs["u1"][:], t1[:, HN:PACK],
                            start=False, stop=False, skip_group_check=True,
                        ),
                        nc.tensor.matmul(
                            bank[:, :], cs["u1z"][:], t1[:, 0:HN],
                            start=False, stop=True,
                        ),
                    )
                fUdt = hsp.tile([128, HN], BF16, tag="fUdt")
                nc.scalar.mul(fUdt[:], pf[:], DT)

                # ---- step loop: squares on ACT, A in PSUM via identity MMs,
                #      G via fused DVE stt, h updated by accumulating MMs.
                pA = pAp.tile([128, HN], F32, tag="pA")
                Gacc = accp.tile([128, HN], BF16, tag="Gacc")
                a_mms = []
                for t in range(STEPS):
                    hsq = hsp.tile([128, HN], BF16, tag="hsq")
                    nc.scalar.square(hsq[:], ph[:])
                    a_mms.append(nc.tensor.matmul(
                        pA[:, :], cs["i128"][:], hsq[:],
                        start=(t == 0), stop=(t == STEPS - 1),
                    ))
                    if t == 0:
                        nc.vector.tensor_scalar(
                            Gacc[:], hsq[:], float(STEPS - 1), None, A.mult,
                        )
                    elif t <= STEPS - 2:
                        nc.vector.scalar_tensor_tensor(
                            out=Gacc[:], in0=hsq[:],
                            scalar=float(STEPS - 1 - t),
                            in1=Gacc[:], op0=A.mult, op1=A.add,
                        )
                    if t < STEPS - 1:
                        for half in range(2):
                            osl = slice(half * 64, (half + 1) * 64)
                            nc.tensor.matmul(
                                ph[osl, :], cs["mdn"][osl, :], hsq[osl, :],
                                start=False, stop=False, skip_group_check=True,
                            )
                            nc.tensor.matmul(
                                ph[osl, :], cs["i64"][osl, :], fUdt[osl, :],
                                start=False, stop=False, skip_group_check=True,
                            )
                _chain(*a_mms)
                At = atp.tile([128, HN], BF16, tag="At")
                nc.scalar.copy(At[:], pA[:])

                # ---- epilogue
                xt = natx.tile([128, NBLK, D], F32, tag="xt")
                nc.sync.dma_start(
                    out=xt[:], in_=xg[rows, :].rearrange("(n p) d -> p n d", p=128)
                )
                vf_sb = outp.tile([128, NBLK, D], F32, tag="vf_sb")
                xf_sb = outp.tile([128, NBLK, D], F32, tag="xf_sb")

                for bg in range(4):      # bank groups: 2 natural blocks each
                    b0, b1 = bg * 2, bg * 2 + 2
                    pvf = pep.tile([128, 2, D], F32, tag="pe")
                    pxf = pep.tile([128, 2, D], F32, tag="pe")
                    vf_mms = []
                    xf_mms = []
                    for j in range(2):
                        blk = bg * 2 + j
                        half = blk // 4
                        hsl = slice(half * 64, (half + 1) * 64)
                        lsl = slice((blk % 4) * 128, (blk % 4) * 128 + 128)
                        vf_mms.append(nc.tensor.matmul(
                            pvf[:, j, :], At[hsl, lsl], cs["wn"][hsl, :],
                            start=(j == 0), stop=False,
                        ))
                        xf_mms.append(nc.tensor.matmul(
                            pxf[:, j, :], Gacc[hsl, lsl], cs["wnn"][hsl, :],
                            start=(j == 0), stop=False,
                        ))
                    vf_mms.append(nc.tensor.matmul(
                        pvf[:, :, :], cs["if8"][:], fb[:, b0:b1, :],
                        start=False, stop=True,
                    ))
                    xf_mms.append(nc.tensor.matmul(
                        pxf[:, :, :], cs["if8"][:], vb[:, b0:b1, :],
                        start=False, stop=False,
                    ))
                    xf_mms.append(nc.tensor.matmul(
                        pxf[:, :, :], cs["i28"][:], fb[:, b0:b1, :],
                        start=False, stop=True,
                    ))
                    _chain(*vf_mms)
                    _chain(*xf_mms)

                    # vf = v0 + (8dt*force + A@Wn)     [exact fp32 add, DVE]
                    nc.vector.tensor_tensor(
                        vf_sb[:, b0:b1, :], vt[:, b0:b1, :], pvf[:], A.add
                    )
                    # q = x0 + (8dt*v0 + 28dt^2*force + G@Wnn)
                    q = wrapp.tile([128, 2, D], F32, tag="q")
                    nc.vector.tensor_tensor(q[:], xt[:, b0:b1, :], pxf[:], A.add)
                    # wrap: r = RNE(q/2pi) via magic const; xf = q - 2pi*r
                    a1 = wrapp.tile([128, 2, D], F32, tag="a1")
                    nc.scalar.activation(
                        out=a1[:], in_=q[:],
                        func=mybir.ActivationFunctionType.Identity,
                        bias=magic_s[:], scale=1.0 / TWO_PI,
                    )
                    rr = wrapp.tile([128, 2, D], BF16, tag="rr")
                    nc.vector.tensor_scalar(
                        rr[:], a1[:], MAGIC, None, A.subtract,
                    )
                    nc.vector.scalar_tensor_tensor(
                        out=xf_sb[:, b0:b1, :], in0=rr[:],
                        scalar=-TWO_PI, in1=q[:], op0=A.mult, op1=A.add,
                    )

                nc.sync.dma_start(
                    out=vo[rows, :].rearrange("(n p) d -> p n d", p=128),
                    in_=vf_sb[:],
                )
                nc.sync.dma_start(
                    out=xo[rows, :].rearrange("(n p) d -> p n d", p=128),
                    in_=xf_sb[:],
                )

    nc.compile()
    return nc


_NC_CACHE = {}


def _get_nc(bl: int):
    if bl not in _NC_CACHE:
        _NC_CACHE[bl] = _build(bl)
    return _NC_CACHE[bl]


def _consts(U, W):
    U32 = np.ascontiguousarray(U, dtype=np.float32)
    W32 = np.ascontiguousarray(W, dtype=np.float32)
    bf = ml_dtypes.bfloat16
    dup = lambda a: np.concatenate([a, a], axis=0)
    md = -(DT * (W32 @ U32))
    eye = np.eye(128, dtype=np.float32)
    z = np.zeros((128, 64), np.float32)
    return {
        "u0z": np.concatenate([U32[:128, :], z], axis=1).astype(bf),
        "u1z": np.concatenate([U32[128:, :], z], axis=1).astype(bf),
        "u0": U32[:128, :].astype(bf),
        "u1": U32[128:, :].astype(bf),
        "mdn": dup(md).astype(bf),
        "i64": dup(np.eye(R, dtype=np.float32)).astype(bf),
        "wn": dup(-DT * W32).astype(bf),
        "wnn": dup(-DT * DT * W32).astype(bf),
        "i128": eye.astype(bf),
        "if8": ((8.0 * DT) * eye).astype(bf),
        "i28": ((28.0 * DT * DT) * eye).astype(bf),
    }


def kernel(x, v, force, U, W, steps=STEPS, **_ignored):
    assert int(steps) == STEPS, f"kernel hardcodes steps={STEPS}, got {steps}"
    x = np.ascontiguousarray(x, dtype=np.float32)
    v = np.ascontiguousarray(v, dtype=np.float32)
    force = np.ascontiguousarray(force, dtype=np.float32)
    consts = _consts(U, W)

    nc = _get_nc(BL)
    in_maps = []
    for i in range(NCORES):
        sl = slice(i * BL, (i + 1) * BL)
        m = {"xg": x[sl], "vg": v[sl], "fg": force[sl]}
        m.update(consts)
        in_maps.append(m)

    res = run_bass_kernel_spmd(nc, in_maps, core_ids=list(range(NCORES)))
    xf = np.concatenate([res.results[i]["xo"] for i in range(NCORES)], axis=0)
    vf = np.concatenate([res.results[i]["vo"] for i in range(NCORES)], axis=0)
    return (xf, vf)



# revision 7
# speedup vs baseline: 1.7434x; 1.7434x over previous
"""Euler integrator (low-rank quadratic Christoffel term) on 8 trn2 NeuronCores.

Math: per step   h = v @ U; gamma = (h*h) @ W; v' = v + dt*(force - gamma);
                 x' = wrap(x + dt*v)
Key reduction: the dynamics close in the rank-64 space:
    h_{t+1} = h_t + dt*(force@U) - (h_t^2) @ (dt*W@U)
    v_T = v_0 + T*dt*force - dt * A @ W,          A = sum_t h_t^2
    x_T = wrap(x_0 + T*dt*v_0 + 28*dt^2*force - dt^2 * G @ W),
                                                  G = sum_t (T-1-t) h_t^2
with T=8.  One wrap at the end (mod-2pi commutes with addition), so HBM
traffic is the 5-tensor roofline.

v2 structure (DMA-dispatch-count bound fix): x/v/force are host-packed into
one [3, bl, D] DRAM tensor and x'/v' into one [2, bl, D] output, so each
1024-row pack is exactly ONE 3MB load + ONE 2MB store on the sync HWDGE
ring.  The rank-64 projection transposes go through the DMA xbar as TWO
batched [128, 2048] bf16 transposes per pack (d-chunk-major staging layout
written by gpsimd), issued on the scalar HWDGE ring so they overlap the HBM
stream.  Step-loop h updates use block-diagonal constants so every PE op has
full 128-contraction.  Running h lives in PSUM, updated purely by
accumulating matmuls; exact fp32 +x0/+v0 adds happen on DVE.
"""

import sys

sys.path.insert(0, "/opt/trn_rl_repo")

import numpy as np
import ml_dtypes

import concourse.bacc as bacc
import concourse.mybir as mybir
import concourse.tile as tile
from concourse.tile_rust import add_dep_helper
from concourse.bass_utils import run_bass_kernel_spmd

F32 = mybir.dt.float32
BF16 = mybir.dt.bfloat16

DT = 0.01
PI = float(np.pi)
TWO_PI = 2.0 * PI
B, D, R = 262144, 256, 64
NCORES = 8
BL = B // NCORES          # rows per core
STEPS = 8
PACK = 1024               # batch rows per pack
NBLK = PACK // 128        # natural 128-row blocks per pack (8)
HN = 512                  # free size of h-space tiles (PACK/2)
MAGIC = 12582912.0        # 1.5 * 2**23 (fp32 RNE rounding trick)


def _chain(*insts):
    for a, b in zip(insts[1:], insts[:-1]):
        add_dep_helper(a.ins, b.ins, sync=True, reason="psum group order")


def _build(bl: int):
    npack = bl // PACK
    nc = bacc.Bacc("TRN2", target_bir_lowering=False, debug=False)

    xvf = nc.declare_dram_parameter("xvf", [npack, 3, PACK, D], F32, isOutput=False)
    # constants (host-prepared, tiny; all bf16 for single-pass matmuls)
    cdefs = {
        "u0z": 128, "u1z": 128,     # [U0|0], [U1|0]
        "u0": R, "u1": R,           # U halves
        "bdmd": 128,                # blockdiag(-dt*(W@U)) twice
        "wn": D, "wnn": D,          # -dt*W, -dt^2*W, dup'd on both halves
        "i128": 128,                # I_128 (A accumulation + fUdt adds)
        "if8": 128, "i28": 128,     # 8dt*I, 28dt^2*I
    }
    cdram = {
        nm: nc.declare_dram_parameter(nm, [128, w], BF16, isOutput=False)
        for nm, w in cdefs.items()
    }
    xvo = nc.declare_dram_parameter("xvo", [npack, 2, PACK, D], F32, isOutput=True)

    A = mybir.AluOpType

    with tile.TileContext(nc) as tc:
        with (
            tc.tile_pool(name="consts", bufs=1) as cpool,
            tc.tile_pool(name="ld", bufs=2) as ldp,
            tc.tile_pool(name="bf", bufs=2) as bfp,
            tc.tile_pool(name="b2", bufs=2) as b2p,
            tc.tile_pool(name="tr", bufs=2) as trp,
            tc.tile_pool(name="hsp", bufs=3) as hsp,
            tc.tile_pool(name="acc", bufs=2) as accp,
            tc.tile_pool(name="atp", bufs=2) as atp,
            tc.tile_pool(name="outp", bufs=2) as outp,
            tc.tile_pool(name="wrapp", bufs=2) as wrapp,
            tc.tile_pool(name="ph", bufs=2, space="PSUM") as php,
            tc.tile_pool(name="pf", bufs=1, space="PSUM") as pfp,
            tc.tile_pool(name="pA", bufs=1, space="PSUM") as pAp,
            tc.tile_pool(name="pe", bufs=3, space="PSUM") as pep,
        ):
            cs = {}
            for nm, w in cdefs.items():
                t_ = cpool.tile([128, w], BF16, tag=nm)
                nc.sync.dma_start(out=t_[:], in_=cdram[nm][:])
                cs[nm] = t_

            for p in range(npack):
                # ---- ONE load for x, v, force (3MB)
                ld = ldp.tile([128, 3, NBLK, D], F32, tag="ld")
                nc.sync.dma_start(
                    out=ld[:],
                    in_=xvf[p].rearrange("t (n q) d -> q t n d", q=128),
                )
                # bf16 casts: natural layout (for epilogue matmuls)
                vb = bfp.tile([128, NBLK, D], BF16, tag="vb")
                fb = bfp.tile([128, NBLK, D], BF16, tag="fb")
                nc.scalar.copy(vb[:], ld[:, 1])
                nc.scalar.copy(fb[:], ld[:, 2])
                # d-chunk-major staging for the batched xbar transpose
                vb2 = b2p.tile([128, 2, NBLK, 128], BF16, tag="vb2")
                fb2 = b2p.tile([128, 2, NBLK, 128], BF16, tag="fb2")
                nc.gpsimd.tensor_copy(
                    out=vb2[:], in_=ld[:, 1].rearrange("p n (c e) -> p c n e", c=2)
                )
                nc.gpsimd.tensor_copy(
                    out=fb2[:], in_=ld[:, 2].rearrange("p n (c e) -> p c n e", c=2)
                )

                # ---- batched transposes: [128, 2048] -> 16 blocks of 128x128
                # vT[:, 0:8, :] == v rows^T for d in 0:128, vT[:, 8:16, :] d 128:256
                vT = trp.tile([128, 2 * NBLK, 128], BF16, tag="vT")
                fT = trp.tile([128, 2 * NBLK, 128], BF16, tag="fT")
                nc.scalar.dma_start(out=vT[:], in_=vb2[:], transpose=True)
                nc.scalar.dma_start(out=fT[:], in_=fb2[:], transpose=True)

                # ---- h0 into persistent psum bank; fU -> fUdt (bf16)
                # layout: bank[0:64, :]  = h rows 0:512 (transposed),
                #         bank[64:128,:] = h rows 512:1024
                ph = php.tile([128, HN], F32, tag="ph")
                pf = pfp.tile([128, HN], F32, tag="pf")
                for bank, src in ((ph, vT), (pf, fT)):
                    _chain(
                        nc.tensor.matmul(
                            bank[:, :], cs["u0z"][:], src[:, 0:4, :],
                            start=True, stop=False,
                        ),
                        nc.tensor.matmul(
                            bank[64:128, :], cs["u0"][:], src[:, 4:8, :],
                            start=False, stop=False, skip_group_check=True,
                        ),
                        nc.tensor.matmul(
                            bank[64:128, :], cs["u1"][:], src[:, 12:16, :],
                            start=False, stop=False, skip_group_check=True,
                        ),
                        nc.tensor.matmul(
                            bank[:, :], cs["u1z"][:], src[:, 8:12, :],
                            start=False, stop=True,
                        ),
                    )
                fUdt = hsp.tile([128, HN], BF16, tag="fUdt")
                nc.scalar.mul(fUdt[:], pf[:], DT)

                # ---- step loop: squares on ACT, A in PSUM via identity MMs,
                #      G via fused DVE stt, h updated by full-128 BD matmuls.
                pA = pAp.tile([128, HN], F32, tag="pA")
                Gacc = accp.tile([128, HN], BF16, tag="Gacc")
                a_mms = []
                for t in range(STEPS):
                    hsq = hsp.tile([128, HN], BF16, tag="hsq")
                    nc.scalar.square(hsq[:], ph[:])
                    a_mms.append(nc.tensor.matmul(
                        pA[:, :], cs["i128"][:], hsq[:],
                        start=(t == 0), stop=(t == STEPS - 1),
                    ))
                    if t == 0:
                        nc.vector.tensor_scalar(
                            Gacc[:], hsq[:], float(STEPS - 1), None, A.mult,
                        )
                    elif t <= STEPS - 2:
                        nc.vector.scalar_tensor_tensor(
                            out=Gacc[:], in0=hsq[:],
                            scalar=float(STEPS - 1 - t),
                            in1=Gacc[:], op0=A.mult, op1=A.add,
                        )
                    if t < STEPS - 1:
                        nc.tensor.matmul(
                            ph[:, :], cs["i128"][:], fUdt[:],
                            start=False, stop=False, skip_group_check=True,
                        )
                        nc.tensor.matmul(
                            ph[:, :], cs["bdmd"][:], hsq[:],
                            start=False, stop=False, skip_group_check=True,
                        )
                _chain(*a_mms)
                At = atp.tile([128, HN], BF16, tag="At")
                nc.scalar.copy(At[:], pA[:])

                # ---- epilogue
                xv = outp.tile([128, 2, NBLK, D], F32, tag="xv")

                for bg in range(4):      # bank groups: 2 natural blocks each
                    b0, b1 = bg * 2, bg * 2 + 2
                    pvf = pep.tile([128, 2, D], F32, tag="pe")
                    pxf = pep.tile([128, 2, D], F32, tag="pe")
                    vf_mms = []
                    xf_mms = []
                    for j in range(2):
                        blk = bg * 2 + j
                        half = blk // 4
                        hsl = slice(half * 64, (half + 1) * 64)
                        lsl = slice((blk % 4) * 128, (blk % 4) * 128 + 128)
                        vf_mms.append(nc.tensor.matmul(
                            pvf[:, j, :], At[hsl, lsl], cs["wn"][hsl, :],
                            start=(j == 0), stop=False,
                        ))
                        xf_mms.append(nc.tensor.matmul(
                            pxf[:, j, :], Gacc[hsl, lsl], cs["wnn"][hsl, :],
                            start=(j == 0), stop=False,
                        ))
                    vf_mms.append(nc.tensor.matmul(
                        pvf[:, :, :], cs["if8"][:], fb[:, b0:b1, :],
                        start=False, stop=True,
                    ))
                    xf_mms.append(nc.tensor.matmul(
                        pxf[:, :, :], cs["if8"][:], vb[:, b0:b1, :],
                        start=False, stop=False,
                    ))
                    xf_mms.append(nc.tensor.matmul(
                        pxf[:, :, :], cs["i28"][:], fb[:, b0:b1, :],
                        start=False, stop=True,
                    ))
                    _chain(*vf_mms)
                    _chain(*xf_mms)

                    # vf = v0 + (8dt*force + A@Wn)     [exact fp32 add, DVE]
                    nc.vector.tensor_tensor(
                        xv[:, 1, b0:b1, :], ld[:, 1, b0:b1, :], pvf[:], A.add
                    )
                    # q = x0 + (8dt*v0 + 28dt^2*force + G@Wnn)
                    q = wrapp.tile([128, 2, D], F32, tag="q")
                    nc.vector.tensor_tensor(q[:], ld[:, 0, b0:b1, :], pxf[:], A.add)
                    # wrap: r = RNE(q/2pi) via magic const; xf = q - 2pi*r
                    a1 = wrapp.tile([128, 2, D], F32, tag="a1")
                    nc.vector.tensor_scalar(
                        a1[:], q[:], 1.0 / TWO_PI, MAGIC, A.mult, A.add,
                    )
                    rr = wrapp.tile([128, 2, D], BF16, tag="rr")
                    nc.vector.tensor_scalar(
                        rr[:], a1[:], MAGIC, None, A.subtract,
                    )
                    nc.vector.scalar_tensor_tensor(
                        out=xv[:, 0, b0:b1, :], in0=rr[:],
                        scalar=-TWO_PI, in1=q[:], op0=A.mult, op1=A.add,
                    )

                # ---- ONE store for x', v' (2MB)
                nc.sync.dma_start(
                    out=xvo[p].rearrange("t (n q) d -> q t n d", q=128),
                    in_=xv[:],
                )

    nc.compile()
    return nc


_NC_CACHE = {}


def _get_nc(bl: int):
    if bl not in _NC_CACHE:
        _NC_CACHE[bl] = _build(bl)
    return _NC_CACHE[bl]


def _consts(U, W):
    U32 = np.ascontiguousarray(U, dtype=np.float32)
    W32 = np.ascontiguousarray(W, dtype=np.float32)
    bf = ml_dtypes.bfloat16
    dup = lambda a: np.concatenate([a, a], axis=0)
    md = -(DT * (W32 @ U32))
    eye = np.eye(128, dtype=np.float32)
    z = np.zeros((128, 64), np.float32)
    zr = np.zeros((R, R), np.float32)
    return {
        "u0z": np.concatenate([U32[:128, :], z], axis=1).astype(bf),
        "u1z": np.concatenate([U32[128:, :], z], axis=1).astype(bf),
        "u0": U32[:128, :].astype(bf),
        "u1": U32[128:, :].astype(bf),
        "bdmd": np.block([[md, zr], [zr, md]]).astype(bf),
        "wn": dup(-DT * W32).astype(bf),
        "wnn": dup(-DT * DT * W32).astype(bf),
        "i128": eye.astype(bf),
        "if8": ((8.0 * DT) * eye).astype(bf),
        "i28": ((28.0 * DT * DT) * eye).astype(bf),
    }


def kernel(x, v, force, U, W, steps=STEPS, **_ignored):
    assert int(steps) == STEPS, f"kernel hardcodes steps={STEPS}, got {steps}"
    x = np.asarray(x, dtype=np.float32)
    v = np.asarray(v, dtype=np.float32)
    force = np.asarray(force, dtype=np.float32)
    consts = _consts(U, W)

    nc = _get_nc(BL)
    npack = BL // PACK
    in_maps = []
    for i in range(NCORES):
        sl = slice(i * BL, (i + 1) * BL)
        # [3, BL, D] -> [npack, 3, PACK, D]: per-pack interleave so each
        # pack's x/v/f rows are one 3D-balanceable DMA.
        stk = np.stack([x[sl], v[sl], force[sl]])
        stk = np.ascontiguousarray(
            stk.reshape(3, npack, PACK, D).transpose(1, 0, 2, 3)
        )
        m = {"xvf": stk}
        m.update(consts)
        in_maps.append(m)

    res = run_bass_kernel_spmd(nc, in_maps, core_ids=list(range(NCORES)))
    xf = np.empty((B, D), np.float32)
    vf = np.empty((B, D), np.float32)
    for i in range(NCORES):
        out = res.results[i]["xvo"]         # [npack, 2, PACK, D]
        sl = slice(i * BL, (i + 1) * BL)
        xf[sl] = out[:, 0].reshape(BL, D)
        vf[sl] = out[:, 1].reshape(BL, D)
    return (xf, vf)


# revision 8
# speedup vs baseline: 1.8513x; 1.0619x over previous
"""Euler integrator (low-rank quadratic Christoffel term) on 8 trn2 NeuronCores.

Math: per step   h = v @ U; gamma = (h*h) @ W; v' = v + dt*(force - gamma);
                 x' = wrap(x + dt*v)
Key reduction: the dynamics close in the rank-64 space:
    h_{t+1} = h_t + dt*(force@U) - (h_t^2) @ (dt*W@U)
    v_T = v_0 + T*dt*force - dt * A @ W,          A = sum_t h_t^2
    x_T = wrap(x_0 + T*dt*v_0 + 28*dt^2*force - dt^2 * G @ W),
                                                  G = sum_t (T-1-t) h_t^2
with T=8, and G == sum of A's running partial sums.  One wrap at the end
(mod-2pi commutes with addition), so HBM traffic is the 5-tensor roofline.

v3 structure: x/v/force host-packed into one [npack, 3, PACK, D] DRAM tensor
(ONE 3MB load per 1024-row pack, sync ring) and x'/v' into one
[npack, 2, PACK, D] output (ONE 2MB store, scalar ring).  ONE fused bf16
cast (v|f) and ONE batched [128, 4096] DMA-xbar transpose per pack (sync
ring) feed the rank-64 projection; the step loop runs entirely on full
128-contraction matmuls (block-diagonal -dt*W@U), with A and G both
accumulated on the tensor engine via (scaled) identity matmuls.  DVE does
only the exact fp32 +x0/+v0 adds and the magic-constant wrap.
"""

import sys

sys.path.insert(0, "/opt/trn_rl_repo")

import numpy as np
import ml_dtypes

import concourse.bacc as bacc
import concourse.mybir as mybir
import concourse.tile as tile
from concourse.tile_rust import add_dep_helper
from concourse.bass_utils import run_bass_kernel_spmd

F32 = mybir.dt.float32
BF16 = mybir.dt.bfloat16

DT = 0.01
PI = float(np.pi)
TWO_PI = 2.0 * PI
B, D, R = 262144, 256, 64
NCORES = 8
BL = B // NCORES          # rows per core
STEPS = 8
PACK = 1024               # batch rows per pack
NBLK = PACK // 128        # natural 128-row blocks per pack (8)
HN = 512                  # free size of h-space tiles (PACK/2)
MAGIC = 12582912.0        # 1.5 * 2**23 (fp32 RNE rounding trick)


def _chain(*insts):
    for a, b in zip(insts[1:], insts[:-1]):
        add_dep_helper(a.ins, b.ins, sync=True, reason="psum group order")


def _build(bl: int):
    npack = bl // PACK
    nc = bacc.Bacc("TRN2", target_bir_lowering=False, debug=False)

    xvf = nc.declare_dram_parameter("xvf", [npack, 3, PACK, D], F32, isOutput=False)
    # constants (host-prepared, tiny; all bf16 for single-pass matmuls)
    cdefs = {
        "u0z": 128, "u1z": 128,     # [U0|0], [U1|0]
        "u0": R, "u1": R,           # U halves
        "bdmd": 128,                # blockdiag(-dt*(W@U)) twice
        "wn": D, "wnn": D,          # -dt*W, -dt^2*W, dup'd on both halves
        "i128": 128,                # I_128 (A accumulation + fUdt adds)
        "if8": 128, "i28": 128,     # 8dt*I, 28dt^2*I
    }
    for t in range(STEPS - 1):
        cdefs[f"ig{t}"] = 128       # (7-t)*I for G accumulation on PE
    cdram = {
        nm: nc.declare_dram_parameter(nm, [128, w], BF16, isOutput=False)
        for nm, w in cdefs.items()
    }
    xvo = nc.declare_dram_parameter("xvo", [npack, 2, PACK, D], F32, isOutput=True)

    A = mybir.AluOpType

    with tile.TileContext(nc) as tc:
        with (
            tc.tile_pool(name="consts", bufs=1) as cpool,
            tc.tile_pool(name="ld", bufs=2) as ldp,
            tc.tile_pool(name="bf", bufs=2) as bfp,
            tc.tile_pool(name="tr", bufs=2) as trp,
            tc.tile_pool(name="hsp", bufs=3) as hsp,
            tc.tile_pool(name="atp", bufs=2) as atp,
            tc.tile_pool(name="outp", bufs=2) as outp,
            tc.tile_pool(name="wrapp", bufs=2) as wrapp,
            tc.tile_pool(name="ph", bufs=1, space="PSUM") as php,
            tc.tile_pool(name="pf", bufs=1, space="PSUM") as pfp,
            tc.tile_pool(name="pA", bufs=1, space="PSUM") as pAp,
            tc.tile_pool(name="pG", bufs=1, space="PSUM") as pGp,
            tc.tile_pool(name="pe", bufs=2, space="PSUM") as pep,
        ):
            cs = {}
            for nm, w in cdefs.items():
                t_ = cpool.tile([128, w], BF16, tag=nm)
                nc.sync.dma_start(out=t_[:], in_=cdram[nm][:])
                cs[nm] = t_

            for p in range(npack):
                # ---- ONE load for x, v, force (3MB), sync ring
                ld = ldp.tile([128, 3, NBLK, D], F32, tag="ld")
                nc.sync.dma_start(
                    out=ld[:],
                    in_=xvf[p].rearrange("t (n q) d -> q t n d", q=128),
                )
                # ONE fused bf16 cast of v|f (natural layout, contiguous)
                vfb = bfp.tile([128, 2, NBLK, D], BF16, tag="vfb")
                nc.scalar.copy(vfb[:], ld[:, 1:3])

                # ---- ONE batched xbar transpose [128, 4096] -> 32 blocks.
                # block t of vfT: tensor T=t//16, blk=(t%16)//2, dch=t%2:
                # vfT[p, t, k] = {v,f}[blk*128 + k, dch*128 + p]
                vfT = trp.tile([128, 4 * NBLK, 128], BF16, tag="vfT")
                nc.sync.dma_start(out=vfT[:], in_=vfb[:], transpose=True)

                # ---- h0 / f@U into psum banks (strided interleaved rhs)
                ph = php.tile([128, HN], F32, tag="ph")
                pf = pfp.tile([128, HN], F32, tag="pf")
                for bank, o in ((ph, 0), (pf, 16)):
                    _chain(
                        nc.tensor.matmul(
                            bank[:, :], cs["u0z"][:], vfT[:, o + 0:o + 8:2, :],
                            start=True, stop=False,
                        ),
                        nc.tensor.matmul(
                            bank[64:128, :], cs["u0"][:], vfT[:, o + 8:o + 16:2, :],
                            start=False, stop=False, skip_group_check=True,
                        ),
                        nc.tensor.matmul(
                            bank[64:128, :], cs["u1"][:], vfT[:, o + 9:o + 16:2, :],
                            start=False, stop=False, skip_group_check=True,
                        ),
                        nc.tensor.matmul(
                            bank[:, :], cs["u1z"][:], vfT[:, o + 1:o + 8:2, :],
                            start=False, stop=True,
                        ),
                    )
                fUdt = hsp.tile([128, HN], BF16, tag="fUdt")
                nc.scalar.mul(fUdt[:], pf[:], DT)

                # ---- step loop: squares on ACT; A, G, and h updates all
                #      as full-128-contraction accumulating matmuls.
                pA = pAp.tile([128, HN], F32, tag="pA")
                pG = pGp.tile([128, HN], F32, tag="pG")
                a_mms = []
                g_mms = []
                for t in range(STEPS):
                    hsq = hsp.tile([128, HN], BF16, tag="hsq")
                    nc.scalar.square(hsq[:], ph[:])
                    a_mms.append(nc.tensor.matmul(
                        pA[:, :], cs["i128"][:], hsq[:],
                        start=(t == 0), stop=(t == STEPS - 1),
                    ))
                    if t < STEPS - 1:
                        g_mms.append(nc.tensor.matmul(
                            pG[:, :], cs[f"ig{t}"][:], hsq[:],
                            start=(t == 0), stop=(t == STEPS - 2),
                        ))
                        nc.tensor.matmul(
                            ph[:, :], cs["i128"][:], fUdt[:],
                            start=False, stop=False, skip_group_check=True,
                        )
                        nc.tensor.matmul(
                            ph[:, :], cs["bdmd"][:], hsq[:],
                            start=False, stop=False, skip_group_check=True,
                        )
                _chain(*a_mms)
                _chain(*g_mms)
                At = atp.tile([128, HN], BF16, tag="At")
                Gt = atp.tile([128, HN], BF16, tag="Gt")
                nc.scalar.copy(At[:], pA[:])
                nc.scalar.copy(Gt[:], pG[:])

                # ---- epilogue (all 2D APs)
                xv = outp.tile([128, 2, NBLK, D], F32, tag="xv")

                for bg in range(4):      # 2 natural blocks per group
                    b0, b1 = bg * 2, bg * 2 + 2
                    pvf = pep.tile([128, 2 * D], F32, tag="pvf")
                    pxf = pep.tile([128, 2 * D], F32, tag="pxf")
                    vf_mms = []
                    xf_mms = []
                    for j in range(2):
                        blk = bg * 2 + j
                        half = blk // 4
                        hsl = slice(half * 64, (half + 1) * 64)
                        lsl = slice((blk % 4) * 128, (blk % 4) * 128 + 128)
                        osl = slice(j * D, (j + 1) * D)
                        vf_mms.append(nc.tensor.matmul(
                            pvf[:, osl], At[hsl, lsl], cs["wn"][hsl, :],
                            start=(j == 0), stop=False,
                        ))
                        xf_mms.append(nc.tensor.matmul(
                            pxf[:, osl], Gt[hsl, lsl], cs["wnn"][hsl, :],
                            start=(j == 0), stop=False,
                        ))
                    vf_mms.append(nc.tensor.matmul(
                        pvf[:, :], cs["if8"][:], vfb[:, 1, b0:b1, :],
                        start=False, stop=True,
                    ))
                    xf_mms.append(nc.tensor.matmul(
                        pxf[:, :], cs["if8"][:], vfb[:, 0, b0:b1, :],
                        start=False, stop=False,
                    ))
                    xf_mms.append(nc.tensor.matmul(
                        pxf[:, :], cs["i28"][:], vfb[:, 1, b0:b1, :],
                        start=False, stop=True,
                    ))
                    _chain(*vf_mms)
                    _chain(*xf_mms)

                    # vf = v0 + (8dt*force + A@Wn)     [exact fp32 add, DVE]
                    nc.vector.tensor_tensor(
                        xv[:, 1, b0:b1, :], ld[:, 1, b0:b1, :], pvf[:], A.add
                    )
                    # q = x0 + (8dt*v0 + 28dt^2*force + G@Wnn)
                    q = wrapp.tile([128, 2 * D], F32, tag="q")
                    nc.vector.tensor_tensor(q[:], ld[:, 0, b0:b1, :], pxf[:], A.add)
                    # wrap: r = RNE(q/2pi) via magic const; xf = q - 2pi*r
                    a1 = wrapp.tile([128, 2 * D], F32, tag="a1")
                    nc.vector.tensor_scalar(
                        a1[:], q[:], 1.0 / TWO_PI, MAGIC, A.mult, A.add,
                    )
                    rr = wrapp.tile([128, 2 * D], BF16, tag="rr")
                    nc.vector.tensor_scalar(
                        rr[:], a1[:], MAGIC, None, A.subtract,
                    )
                    nc.vector.scalar_tensor_tensor(
                        out=xv[:, 0, b0:b1, :], in0=rr[:],
                        scalar=-TWO_PI, in1=q[:], op0=A.mult, op1=A.add,
                    )

                # ---- ONE store for x', v' (2MB), scalar ring
                nc.scalar.dma_start(
                    out=xvo[p].rearrange("t (n q) d -> q t n d", q=128),
                    in_=xv[:],
                )

    nc.compile()
    return nc


_NC_CACHE = {}


def _get_nc(bl: int):
    if bl not in _NC_CACHE:
        _NC_CACHE[bl] = _build(bl)
    return _NC_CACHE[bl]


def _consts(U, W):
    U32 = np.ascontiguousarray(U, dtype=np.float32)
    W32 = np.ascontiguousarray(W, dtype=np.float32)
    bf = ml_dtypes.bfloat16
    dup = lambda a: np.concatenate([a, a], axis=0)
    md = -(DT * (W32 @ U32))
    eye = np.eye(128, dtype=np.float32)
    z = np.zeros((128, 64), np.float32)
    zr = np.zeros((R, R), np.float32)
    out = {
        "u0z": np.concatenate([U32[:128, :], z], axis=1).astype(bf),
        "u1z": np.concatenate([U32[128:, :], z], axis=1).astype(bf),
        "u0": U32[:128, :].astype(bf),
        "u1": U32[128:, :].astype(bf),
        "bdmd": np.block([[md, zr], [zr, md]]).astype(bf),
        "wn": dup(-DT * W32).astype(bf),
        "wnn": dup(-DT * DT * W32).astype(bf),
        "i128": eye.astype(bf),
        "if8": ((8.0 * DT) * eye).astype(bf),
        "i28": ((28.0 * DT * DT) * eye).astype(bf),
    }
    for t in range(STEPS - 1):
        out[f"ig{t}"] = (float(STEPS - 1 - t) * eye).astype(bf)
    return out


def kernel(x, v, force, U, W, steps=STEPS, **_ignored):
    assert int(steps) == STEPS, f"kernel hardcodes steps={STEPS}, got {steps}"
    x = np.asarray(x, dtype=np.float32)
    v = np.asarray(v, dtype=np.float32)
    force = np.asarray(force, dtype=np.float32)
    consts = _consts(U, W)

    nc = _get_nc(BL)
    npack = BL // PACK
    in_maps = []
    for i in range(NCORES):
        sl = slice(i * BL, (i + 1) * BL)
        # [3, BL, D] -> [npack, 3, PACK, D]: per-pack interleave so each
        # pack's x/v/f rows are one 3D-balanceable DMA.
        stk = np.stack([x[sl], v[sl], force[sl]])
        stk = np.ascontiguousarray(
            stk.reshape(3, npack, PACK, D).transpose(1, 0, 2, 3)
        )
        m = {"xvf": stk}
        m.update(consts)
        in_maps.append(m)

    res = run_bass_kernel_spmd(nc, in_maps, core_ids=list(range(NCORES)))
    xf = np.empty((B, D), np.float32)
    vf = np.empty((B, D), np.float32)
    for i in range(NCORES):
        out = res.results[i]["xvo"]         # [npack, 2, PACK, D]
        sl = slice(i * BL, (i + 1) * BL)
        xf[sl] = out[:, 0].reshape(BL, D)
        vf[sl] = out[:, 1].reshape(BL, D)
    return (xf, vf)
